# revision 20
# baseline (speedup 1.0000x reference)
"""Causal self-attention Trainium2 kernel (B=8, T=1024, C=768, H=12 heads).

Strategy: data-parallel over batch — one batch element per NeuronCore (8 cores).
Per core, everything is computed in a "transposed" layout so that no on-device
transposes are needed:

  qT, kT  [C, T]   = w_attn_{q,k}.T @ x.T          (x.T supplied by host)
  v_aug   [T, 781] = x @ [w_attn_v | 0]  (+ ones column per head, stride 65)
  sT_h    [Tk, Tq] = kT_h.T-slices @ qT_h          (keys on partitions)
  eT      = exp(sT / 8) with causal mask (memset + triangular multiplicative)
  yT_aug  [65, Tq] = v_aug_h.T @ eT                (row 64 = softmax row-sums)
  yT_norm = yT * broadcast(1/sums)                 (broadcast via one-hot matmul)
  out     [T, C]   = yT_norm.T-slices @ w_proj     (DMA PSUM -> DRAM)

All matmuls run as float32r (reduced-precision fp32 PE mode, ~1.5e-4 rel err,
4x the fp32 throughput at free-dim >= 256).
"""
import sys

sys.path.insert(0, "/opt/trn_rl_repo")

import numpy as np

import concourse.bacc as bacc
import concourse.tile as tile
import concourse.mybir as mybir
from concourse.bass_utils import run_bass_kernel_spmd

f32 = mybir.dt.float32
f32r = mybir.dt.float32r
EXP = mybir.ActivationFunctionType.Exp

B, T, C = 8, 1024, 768
H, D = 12, 64
DA = D + 1  # head stride in v_aug (extra ones column)
NK = C // 128  # 6 contraction tiles
NT = T // 128  # 8 token tiles
SCALE = 1.0 / np.sqrt(D)


def build():
    nc = bacc.Bacc("TRN2", target_bir_lowering=False, debug=False)
    xT = nc.dram_tensor("xT", [C, T], f32r, kind="ExternalInput")
    wq = nc.dram_tensor("wq", [2 * NK, 128, NK, 128], f32r, kind="ExternalInput")
    wv = nc.dram_tensor("wv", [NK, 128, H * DA], f32r, kind="ExternalInput")
    wp = nc.dram_tensor("wp", [NK, 128, C], f32r, kind="ExternalInput")
    msk = nc.dram_tensor("msk", [4, 128, 512], f32r, kind="ExternalInput")
    onesc = nc.dram_tensor("onesc", [128, H], f32r, kind="ExternalInput")
    sel = nc.dram_tensor("sel", [H, C], f32r, kind="ExternalInput")
    out = nc.dram_tensor("out", [T, C], f32, kind="ExternalOutput")

    with tile.TileContext(nc) as tc:
        with (
            tc.tile_pool(name="const", bufs=1) as const,
            tc.tile_pool(name="wqp", bufs=2) as wqp,
            tc.tile_pool(name="exp", bufs=4) as expp,
            tc.tile_pool(name="psc", bufs=3, space="PSUM") as psc,
            tc.tile_pool(name="psm", bufs=2, space="PSUM") as psm,
        ):
            # ---- resident SBUF tensors ----
            xT_t = [const.tile([128, T], f32r, name=f"xTs{i}", tag=f"xT{i}") for i in range(NK)]
            wv_t = [const.tile([128, H * DA], f32r, name=f"wvs{i}", tag=f"wv{i}") for i in range(NK)]
            wp_t = [const.tile([128, C], f32r, name=f"wps{i}", tag=f"wp{i}") for i in range(NK)]
            qkT_t = [const.tile([128, T], f32r, name=f"qks{m}", tag=f"qk{m}") for m in range(2 * NK)]
            v_t = [const.tile([128, H * DA], f32r, name=f"vs{t}", tag=f"v{t}") for t in range(NT)]
            yT_t = [const.tile([128, T], f32r, name=f"yTs{i}", tag=f"yT{i}") for i in range(NK)]
            msk_t = [const.tile([128, 512], f32r, name=f"msks{i}", tag=f"msk{i}") for i in range(4)]
            ones_t = const.tile([128, H], f32r, tag="ones")
            sel_t = const.tile([H, C], f32r, tag="sel")
            sums_t = const.tile([H, T], f32, tag="sums")
            rec_t = const.tile([H, T], f32r, tag="rec")

            # phase-1a inputs first (halves, so the first matmuls start sooner)
            for i in range(NK):
                nc.sync.dma_start(out=xT_t[i][:, 0:512], in_=xT[i * 128:(i + 1) * 128, 0:512])

            def qk_tile(m, wq_t=None):
                if wq_t is None:
                    wq_t = wqp.tile([128, NK, 128], f32r, tag="wq", name="wq_t")
                    nc.sync.dma_start(out=wq_t, in_=wq[m, :, :, :])
                ps = psc.tile([128, 1024], f32, tag="sc", name="psqk")
                for qc in range(2):
                    for kk in range(NK):
                        nc.tensor.matmul(
                            ps[:, qc * 512:(qc + 1) * 512],
                            wq_t[:, kk, :],
                            xT_t[kk][:, qc * 512:(qc + 1) * 512],
                            start=(kk == 0),
                            stop=(kk == NK - 1),
                        )
                nc.vector.tensor_copy(qkT_t[m], ps)

            def v_tile(t):
                ps = psc.tile([128, 1024], f32, tag="sc", name="psv")
                for n0, nw in ((0, 512), (512, H * DA - 512)):
                    for kk in range(NK):
                        nc.tensor.matmul(
                            ps[:, n0:n0 + nw],
                            xT_t[kk][:, t * 128:(t + 1) * 128],
                            wv_t[kk][:, n0:n0 + nw],
                            start=(kk == 0),
                            stop=(kk == NK - 1),
                        )
                nc.vector.tensor_copy(v_t[t], ps[:, :H * DA])
                ones_ap = v_t[t].rearrange("p (h e) -> p h e", e=DA)[:, :, D]
                nc.vector.tensor_copy(ones_ap, ones_t)

            def attention(hp, qc):
                qs = slice(qc * 512, (qc + 1) * 512)
                nkt = 4 * (qc + 1)
                qT = qkT_t[hp]
                kT = qkT_t[NK + hp]
                ypA = psm.tile([128, 512], f32, tag="mm", name="ypA")
                ypB = psm.tile([128, 512], f32, tag="mm", name="ypB")
                exs = {}
                # software pipeline: attv trails scores/exp by one kt
                for kt in range(nkt + 1):
                    if kt < nkt:
                        ks = slice(kt * 128, (kt + 1) * 128)
                        sp = psc.tile([128, 1024], f32, tag="sc", name="sp")
                        nc.tensor.matmul(
                            sp[:, 0:512], kT[0:64, ks], qT[0:64, qs],
                            start=True, stop=True,
                        )
                        nc.tensor.matmul(
                            sp[:, 512:1024], kT[64:128, ks], qT[64:128, qs],
                            start=True, stop=True,
                        )
                        ex = expp.tile([128, 1024], f32r, tag="ex", bufs=3, name="ex")
                        nc.scalar.activation(ex, sp, EXP, scale=float(SCALE))
                        pos = kt * 128 - qc * 512
                        if pos >= 0:
                            mk = msk_t[pos // 128]
                            nc.vector.tensor_mul(ex[:, 0:512], ex[:, 0:512], mk)
                            nc.vector.tensor_mul(ex[:, 512:1024], ex[:, 512:1024], mk)
                        exs[kt] = ex
                    if kt >= 1:
                        pk = kt - 1
                        exp_ = exs.pop(pk)
                        for h, yp, half in ((2 * hp, ypA, 0), (2 * hp + 1, ypB, 1)):
                            nc.tensor.matmul(
                                yp[:DA, :],
                                v_t[pk][:, h * DA:(h + 1) * DA],
                                exp_[:, half * 512:(half + 1) * 512],
                                start=(pk == 0), stop=(pk == nkt - 1),
                            )
                for h, yp, off in ((2 * hp, ypA, 0), (2 * hp + 1, ypB, 64)):
                    stage = expp.tile([DA, 512], f32r, tag="ystage", bufs=2, name="stage")
                    nc.vector.tensor_copy(stage, yp[:DA, :])
                    nc.sync.dma_start(out=yT_t[hp][off:off + 64, qs], in_=stage[:D, :])
                    nc.sync.dma_start(
                        out=sums_t[h:h + 1, qs], in_=stage[D:DA, :].bitcast(f32)
                    )

            def normalize(qc):
                qs = slice(qc * 512, (qc + 1) * 512)
                with nc.allow_low_precision(reason="f32r recip feeds f32r matmul"):
                    nc.vector.reciprocal(rec_t[:, qs], sums_t[:, qs])
                for hp in range(NK):
                    bc = psc.tile([128, 512], f32, tag="sc", name="bc")
                    nc.tensor.matmul(
                        bc, sel_t[:, hp * 128:(hp + 1) * 128], rec_t[:, qs],
                        start=True, stop=True,
                    )
                    nc.vector.tensor_mul(yT_t[hp][:, qs], yT_t[hp][:, qs], bc.bitcast(f32r))

            def project(qc):
                for t in range(qc * 4, qc * 4 + 4):
                    pp = psc.tile([128, 1024], f32, tag="sc", name="pp")
                    for n0, nw in ((0, 512), (512, 256)):
                        for kk in range(NK):
                            nc.tensor.matmul(
                                pp[:, n0:n0 + nw],
                                yT_t[kk][:, t * 128:(t + 1) * 128],
                                wp_t[kk][:, n0:n0 + nw],
                                start=(kk == 0),
                                stop=(kk == NK - 1),
                            )
                    ostage = expp.tile([128, C], f32, tag="ostage", bufs=2, name="ostage")
                    nc.scalar.copy(ostage, pp[:, :C])
                    nc.sync.dma_start(out=out[t * 128:(t + 1) * 128, :], in_=ostage)

            # ---- interleaved schedule ----
            wq0 = wqp.tile([128, NK, 128], f32r, tag="wq", name="wq0")
            nc.sync.dma_start(out=wq0, in_=wq[0, :, :, :])
            wq6 = wqp.tile([128, NK, 128], f32r, tag="wq", name="wq6")
            nc.sync.dma_start(out=wq6, in_=wq[6, :, :, :])
            for i in range(NK):
                nc.sync.dma_start(
                    out=xT_t[i][:, 512:1024], in_=xT[i * 128:(i + 1) * 128, 512:1024]
                )
            for hp in range(NK):
                qk_tile(hp, wq0 if hp == 0 else None)
                qk_tile(NK + hp, wq6 if hp == 0 else None)
                if hp == 0:
                    for i in range(NK):
                        nc.sync.dma_start(out=wv_t[i], in_=wv[i, :, :])
                    nc.sync.dma_start(out=ones_t, in_=onesc[:, :])
                    for i in range(4):
                        nc.sync.dma_start(out=msk_t[i], in_=msk[i, :, :])
                    nc.sync.dma_start(out=sel_t, in_=sel[:, :])
                    for t in range(4):
                        v_tile(t)
                attention(hp, 0)
                if hp == 2:
                    for i in range(NK):
                        nc.sync.dma_start(out=wp_t[i], in_=wp[i, :, :])
                if hp == 5:
                    for t in range(4, NT):
                        v_tile(t)
            normalize(0)
            project(0)
            for hp in range(NK):
                attention(hp, 1)
            normalize(1)
            project(1)

    nc.compile()
    return nc


_nc = None


def _get_nc():
    global _nc
    if _nc is None:
        _nc = build()
    return _nc


def _host_prep(w_attn, w_proj):
    wq = np.ascontiguousarray(
        w_attn[:, :2 * C].reshape(NK, 128, 2 * NK, 128).transpose(2, 1, 0, 3)
    )
    wv_aug = np.zeros((C, H, DA), np.float32)
    wv_aug[:, :, :D] = w_attn[:, 2 * C:].reshape(C, H, D)
    wv = np.ascontiguousarray(wv_aug.reshape(NK, 128, H * DA))
    wp = np.ascontiguousarray(w_proj.reshape(NK, 128, C))
    msk = np.zeros((4, 128, 512), np.float32)
    for i in range(4):
        for k in range(128):
            msk[i, k, k + i * 128:] = 1.0
    onesc = np.ones((128, H), np.float32)
    sel = np.zeros((H, C), np.float32)
    for p in range(C):
        sel[2 * (p // 128) + (p % 128) // 64, p] = 1.0
    return wq, wv, wp, msk, onesc, sel


def kernel(x, w_attn, w_proj):
    x = np.asarray(x, dtype=np.float32)
    w_attn = np.asarray(w_attn, dtype=np.float32)
    w_proj = np.asarray(w_proj, dtype=np.float32)
    wq, wv, wp, msk, onesc, sel = _host_prep(w_attn, w_proj)
    in_maps = [
        {
            "xT": np.ascontiguousarray(x[b].T),
            "wq": wq,
            "wv": wv,
            "wp": wp,
            "msk": msk,
            "onesc": onesc,
            "sel": sel,
        }
        for b in range(B)
    ]
    res = run_bass_kernel_spmd(_get_nc(), in_maps, list(range(B)))
    return np.stack([res.results[b]["out"] for b in range(B)], axis=0)
